# revision 6
# baseline (speedup 1.0000x reference)
"""Multi-head attention (B=2, S=2048, EMB=1024, 16 heads) on 8 Trainium2 cores.

Sharding: core c -> batch c//4, head-group c%4 (4 heads = 256 projection dims).
Each core computes Q/K projections in transposed layout (head-dim on
partitions), V natural, attention without max subtraction (scores ~ N(0,1) in
fp32), the softmax denominator via a ones-column appended to V (free inside
the ctx matmul M=65), and a row-parallel partial of the output projection.
The host sums the 4 partials per batch and adds the output bias.

HW-measured matmul design rules (see microbenches):
  - back-to-back matmuls on the SAME PE row group cost ~449ns (LDWEIGHTS
    serializes); alternating row groups or full-K with bank alternation gets
    200-255ns (LDW hides behind the other stream).
  - accumulation chains into the same PSUM bank back-to-back cost ~365ns;
    interleaving two chains on different banks avoids the hazard.
So every matmul stream below interleaves two chains (alternating PSUM banks)
and, for K=64 score matmuls, alternates row groups via the hi (head) index.

The exp runs as [128, 1024] activations over 2-bank PSUM score tiles to
amortize the ~143ns per-instruction ACT overhead (ACT is ~128us total and
near-critical).
"""

import numpy as np

import concourse.tile as tile
from concourse import bacc, mybir
from concourse import bass_utils

EMB = 1024
S = 2048
B = 2
HPC = 4            # heads per core
DQ = HPC * 64      # 256 projection dims per core
NCORES = 8

F32 = mybir.dt.float32
F32R = mybir.dt.float32r
EXP = mybir.ActivationFunctionType.Exp

KT_E = EMB // 128  # 8 contraction tiles over EMB
NQC = S // 512     # 4 query chunks
NST = S // 128     # 16 sequence tiles

_NC = None
TRACE = False
LAST_RESULT = None


def _mha(ctx, tc, xqT, xkT, xvT, wqT, wkT, wvT, woT, bq, bk, bv, out, bench_iters=None):
    nc = tc.nc

    cstp = ctx.enter_context(tc.tile_pool(name="const", bufs=1))
    xpool = ctx.enter_context(tc.tile_pool(name="xin", bufs=16))
    epool = ctx.enter_context(tc.tile_pool(name="exp", bufs=4))
    bpool = ctx.enter_context(tc.tile_pool(name="bcsb", bufs=2))
    opool = ctx.enter_context(tc.tile_pool(name="osb", bufs=3))
    upool = ctx.enter_context(tc.tile_pool(name="unorm", bufs=8))
    # PSUM: mpool 2x[128,512] (2 banks) + spool 2x[128,1024] (4 banks)
    #       + cpool 2x[65,512] (2 banks) = 8 banks exactly.
    mpool = ctx.enter_context(tc.tile_pool(name="mmps", bufs=2, space="PSUM"))
    spool = ctx.enter_context(tc.tile_pool(name="scps", bufs=2, space="PSUM"))
    ctx_ps = ctx.enter_context(tc.tile_pool(name="ctxps", bufs=2, space="PSUM"))

    # ---- persistent SBUF tensors ----
    ones_row = cstp.tile([1, 512], F32R)
    nc.vector.memset(ones_row[:].bitcast(F32), 1.0)
    sel64 = cstp.tile([65, 64], F32R)           # one-hot: row 64 -> all cols
    nc.vector.memset(sel64[:].bitcast(F32), 0.0)
    nc.vector.memset(sel64[64:65, :].bitcast(F32), 1.0)
    rden = cstp.tile([65, 512], F32R)           # row 64 = 1/denom, rows 0..63 zero
    nc.vector.memset(rden[0:64, :].bitcast(F32), 0.0)

    wq_sb = cstp.tile([128, KT_E * DQ], F32R)   # [128, 2048]: wq_sb[p, n*256+m] = WqT[n*128+p, m]
    wk_sb = cstp.tile([128, KT_E * DQ], F32R)
    wv_sb = cstp.tile([128, KT_E * DQ], F32R)
    for sb, src in ((wq_sb, wqT), (wk_sb, wkT), (wv_sb, wvT)):
        nc.sync.dma_start(
            sb[:].rearrange("p (n m) -> p n m", n=KT_E),
            src.rearrange("(n p) m -> p n m", p=128),
        )
    wo_sb = cstp.tile([128, 2 * EMB], F32R)     # wo_sb[p, n*1024+f] = WoT[n*128+p, f]
    nc.sync.dma_start(
        wo_sb[:].rearrange("p (n m) -> p n m", n=2),
        woT.rearrange("(n p) m -> p n m", p=128),
    )
    # per-partition bias columns: b?c[p, dq] = bias[dq*128+p]
    bqc = cstp.tile([128, 2], F32)
    bkc = cstp.tile([128, 2], F32)
    for sb, src in ((bqc, bq), (bkc, bk)):
        nc.sync.dma_start(sb[:], src.rearrange("o (d p) -> p (o d)", p=128))
    bv_sb = cstp.tile([1, DQ], F32R)
    nc.sync.dma_start(bv_sb[:], bv[:])

    # results of phase 1 kept resident
    kT_sb = cstp.tile([128, 2 * S], F32R)       # [dq-block 2][s 2048]
    qT_sb = cstp.tile([128, 2 * S], F32R)
    ctxT_sb = cstp.tile([128, 2 * S], F32R)
    v_sb = cstp.tile([128, NST * (HPC * 65)], F32R)  # per s-tile: 4 heads x (64 V + ones col)
    nc.vector.memset(
        v_sb[:].bitcast(F32).rearrange("p (t h m) -> p t h m", t=NST, h=HPC)[:, :, :, 64:65],
        1.0,
    )

    def body():
        _body(tc, nc, xqT, xkT, xvT, out, ones_row, sel64, rden, wq_sb, wk_sb,
              wv_sb, wo_sb, bqc, bkc, bv_sb, kT_sb, qT_sb, ctxT_sb, v_sb,
              xpool, epool, bpool, opool, upool, mpool, spool, ctx_ps)

    if bench_iters:
        hints = (
            mybir.EngineType.PE,
            mybir.EngineType.Activation,
            mybir.EngineType.DVE,
            mybir.EngineType.SP,
            mybir.EngineType.Pool,
        )
        with tc.For_i(0, bench_iters, 1, hint_engines=hints):
            body()
    else:
        body()


def _body(tc, nc, xqT, xkT, xvT, out, ones_row, sel64, rden, wq_sb, wk_sb,
          wv_sb, wo_sb, bqc, bkc, bv_sb, kT_sb, qT_sb, ctxT_sb, v_sb,
          xpool, epool, bpool, opool, upool, mpool, spool, ctx_ps):
    pending_norm = []

    def _norm_and_outproj(qc_done):
        _finish_chunk(nc, qc_done, pending_norm, rden, sel64, bpool,
                      opool, mpool, ctxT_sb, wo_sb, out)

    # ---- phase 1: per sequence chunk, produce Q^T, K^T and V ----
    for qc in range(NQC):
        for pj, (w_sb, xsrc, dst_sb, bcol) in enumerate((
            (wq_sb, xqT, qT_sb, bqc),
            (wk_sb, xkT, kT_sb, bkc),
        )):
            xs = []
            for kt in range(KT_E):
                t = xpool.tile([128, 512], F32R, tag="xchunk", name=f"x_{qc}_{pj}_{kt}")
                nc.sync.dma_start(t[:], xsrc[kt, qc])
                xs.append(t)
            # two dq chains interleaved -> PSUM banks alternate
            ps = [mpool.tile([128, 512], F32, tag="mm", name=f"p_{qc}_{pj}_{dq}")
                  for dq in range(2)]
            for kt in range(KT_E):
                for dq in range(2):
                    nc.tensor.matmul(
                        ps[dq][:], w_sb[:, kt * DQ + dq * 128: kt * DQ + dq * 128 + 128],
                        xs[kt][:], start=(kt == 0), stop=(kt == KT_E - 1),
                    )
            for dq in range(2):
                nc.vector.tensor_scalar_add(
                    dst_sb[:, dq * S + qc * 512: dq * S + qc * 512 + 512],
                    ps[dq][:], bcol[:, dq: dq + 1],
                )
        # V: natural layout, two chains at a time (padded [128,512] psum tiles
        # so the "mm" tag keeps a single slot size)
        xv = []
        for kt in range(KT_E):
            t = xpool.tile([128, 512], F32R, tag="xchunk", name=f"xv_{qc}_{kt}")
            nc.sync.dma_start(t[:], xvT[kt, qc])
            xv.append(t)
        for pair in range(2):
            vp = [mpool.tile([128, 512], F32, tag="mm", name=f"vps_{qc}_{pair}_{i}")
                  for i in range(2)]
            for kt in range(KT_E):
                for i in range(2):
                    sti = pair * 2 + i
                    nc.tensor.matmul(
                        vp[i][:, 0:DQ], xv[kt][:, sti * 128: sti * 128 + 128],
                        wv_sb[:, kt * DQ: kt * DQ + DQ],
                        start=(kt == 0), stop=False,
                    )
            for i in range(2):
                nc.tensor.matmul(
                    vp[i][:, 0:DQ], ones_row[0:1, 0:128], bv_sb[0:1, :],
                    start=False, stop=True,
                )
            for i in range(2):
                sti = pair * 2 + i
                st = qc * 4 + sti
                dst = v_sb[:, st * (HPC * 65): (st + 1) * (HPC * 65)]
                nc.vector.tensor_copy(
                    dst.rearrange("p (h m) -> p h m", h=HPC)[:, :, 0:64],
                    vp[i][:, 0:DQ].rearrange("p (h m) -> p h m", h=HPC),
                )

    # ---- phase 2: per query chunk: attention, then deferred norm/out-proj ----
    for qc in range(NQC):
        for hp in range(2):
            # two ctx accumulation chains (one per head of the pair), K=128,
            # interleaved so consecutive ctx matmuls alternate PSUM banks.
            cps = [ctx_ps.tile([65, 512], F32, tag="ctx", name=f"ctx_{qc}_{hp}_{hi}")
                   for hi in range(2)]
            for kg in range(NST // 2):
                sc2 = [spool.tile([128, 1024], F32, tag="sc", name=f"sc_{qc}_{hp}_{kg}_{hi}")
                       for hi in range(2)]
                for ki in range(2):
                    kt = 2 * kg + ki
                    for hi in range(2):
                        base = 64 * hi
                        blk = hp * S
                        nc.tensor.matmul(
                            sc2[hi][:, ki * 512: ki * 512 + 512],
                            kT_sb[base:base + 64, blk + kt * 128: blk + kt * 128 + 128],
                            qT_sb[base:base + 64, blk + qc * 512: blk + qc * 512 + 512],
                            start=True, stop=True,
                        )
                es = []
                for hi in range(2):
                    e2 = epool.tile([128, 1024], F32R, tag="e", name=f"e_{qc}_{hp}_{kg}_{hi}")
                    nc.scalar.activation(e2[:], sc2[hi][:], EXP, scale=0.125)
                    es.append(e2)
                for ki in range(2):
                    kt = 2 * kg + ki
                    for hi in range(2):
                        h = hp * 2 + hi
                        vcol = kt * (HPC * 65) + h * 65
                        nc.tensor.matmul(
                            cps[hi][:], v_sb[:, vcol: vcol + 65],
                            es[hi][:, ki * 512: ki * 512 + 512],
                            start=(kt == 0), stop=(kt == NST - 1),
                        )
            for hi in range(2):
                # drain PSUM to SBUF; normalization + out-projection are
                # deferred one qc so they overlap the next chunk's attention.
                u = upool.tile([65, 512], F32, tag="u", name=f"u_{qc}_{hp}_{hi}")
                nc.vector.tensor_copy(u[:], cps[hi][:])
                pending_norm.append((qc, hp, hi, u))

        if qc > 0:
            _norm_and_outproj(qc - 1)
    _norm_and_outproj(NQC - 1)


def _finish_chunk(nc, qc, pending_norm, rden, sel64, bpool, opool, mpool,
                  ctxT_sb, wo_sb, out):
    for pqc, hp, hi, u in [p for p in pending_norm if p[0] == qc]:
        # reciprocal of the raw denominator row, broadcast to 64 partitions
        # via one-hot matmul, then multiply (normalizes ctx into ctxT_sb)
        with nc.allow_low_precision(reason="f32r is 32-bit; rounding only"):
            nc.vector.reciprocal(rden[64:65, :], u[64:65, :])
        bps = mpool.tile([64, 512], F32, tag="mm", name=f"bc_{qc}_{hp}_{hi}")
        nc.tensor.matmul(bps[:], sel64[:], rden[:], start=True, stop=True)
        nc.vector.tensor_mul(
            ctxT_sb[64 * hi: 64 * hi + 64, hp * S + qc * 512: hp * S + qc * 512 + 512],
            u[0:64, :],
            bps[:],
        )
    # out projection for this chunk's 4 query tiles; fc chains interleaved
    for qt4 in range(4):
        qt = qc * 4 + qt4
        ps = [mpool.tile([128, 512], F32, tag="mm", name=f"ops_{qt}_{fc}")
              for fc in range(2)]
        for dq in range(2):
            for fc in range(2):
                nc.tensor.matmul(
                    ps[fc][:],
                    ctxT_sb[:, dq * S + qt * 128: dq * S + qt * 128 + 128],
                    wo_sb[:, dq * EMB + fc * 512: dq * EMB + fc * 512 + 512],
                    start=(dq == 0), stop=(dq == 1),
                )
        ot = opool.tile([128, EMB], F32, tag="o", name=f"ot_{qt}")
        for fc in range(2):
            nc.vector.tensor_copy(ot[:, fc * 512: fc * 512 + 512], ps[fc][:])
        nc.gpsimd.dma_start(out[qt * 128:(qt + 1) * 128, :], ot[:])


def _build_nc(bench_iters=None):
    from contextlib import ExitStack

    nc = bacc.Bacc("TRN2", target_bir_lowering=False, debug=False, num_devices=NCORES)
    xqT = nc.dram_tensor("xqT", [KT_E, NQC, 128, 512], F32R, kind="ExternalInput").ap()
    xkT = nc.dram_tensor("xkT", [KT_E, NQC, 128, 512], F32R, kind="ExternalInput").ap()
    xvT = nc.dram_tensor("xvT", [KT_E, NQC, 128, 512], F32R, kind="ExternalInput").ap()
    wqT = nc.dram_tensor("wqT", [EMB, DQ], F32R, kind="ExternalInput").ap()
    wkT = nc.dram_tensor("wkT", [EMB, DQ], F32R, kind="ExternalInput").ap()
    wvT = nc.dram_tensor("wvT", [EMB, DQ], F32R, kind="ExternalInput").ap()
    woT = nc.dram_tensor("woT", [DQ, EMB], F32R, kind="ExternalInput").ap()
    bq = nc.dram_tensor("bq", [1, DQ], F32, kind="ExternalInput").ap()
    bk = nc.dram_tensor("bk", [1, DQ], F32, kind="ExternalInput").ap()
    bv = nc.dram_tensor("bv", [1, DQ], F32R, kind="ExternalInput").ap()
    out = nc.dram_tensor("out", [S, EMB], F32, kind="ExternalOutput").ap()

    with ExitStack() as ctx:
        tc = ctx.enter_context(tile.TileContext(nc))
        _mha(ctx, tc, xqT, xkT, xvT, wqT, wkT, wvT, woT, bq, bk, bv, out,
             bench_iters=bench_iters)
    nc.compile()
    return nc


def _chunk_major(x):
    """[S, EMB] -> x.T chunked as [KT_E, NQC, 128, 512] (each chunk contiguous)."""
    xt = x.T  # [EMB, S]
    return np.ascontiguousarray(
        xt.reshape(KT_E, 128, NQC, 512).transpose(0, 2, 1, 3)
    )


def kernel(query, key, value, Wq, bq, Wk, bk, Wv, bv, Wo, bo):
    global _NC, LAST_RESULT
    query, key, value, Wq, bq, Wk, bk, Wv, bv, Wo, bo = (
        np.asarray(a, dtype=np.float32)
        for a in (query, key, value, Wq, bq, Wk, bk, Wv, bv, Wo, bo)
    )
    if _NC is None:
        _NC = _build_nc()

    in_maps = []
    for c in range(NCORES):
        b, g = divmod(c, 4)
        rows = slice(g * DQ, (g + 1) * DQ)
        in_maps.append({
            "xqT": _chunk_major(query[b]),
            "xkT": _chunk_major(key[b]),
            "xvT": _chunk_major(value[b]),
            "wqT": np.ascontiguousarray(Wq[rows].T),
            "wkT": np.ascontiguousarray(Wk[rows].T),
            "wvT": np.ascontiguousarray(Wv[rows].T),
            "woT": np.ascontiguousarray(Wo[:, rows].T),
            "bq": np.ascontiguousarray(bq[rows][None, :]),
            "bk": np.ascontiguousarray(bk[rows][None, :]),
            "bv": np.ascontiguousarray(bv[rows][None, :]),
        })

    res = bass_utils.run_bass_kernel_spmd(
        _NC, in_maps, core_ids=list(range(NCORES)), trace=TRACE
    )
    LAST_RESULT = res

    out = np.zeros((B, S, EMB), np.float32)
    for c in range(NCORES):
        out[c // 4] += res.results[c]["out"]
    out += bo[None, None, :]
    return out


# revision 10
# speedup vs baseline: 1.1559x; 1.1559x over previous
"""Multi-head attention (B=2, S=2048, EMB=1024, 16 heads) on 8 Trainium2 cores.

Sharding: core c -> batch c//4, head-group c%4 (4 heads = 256 projection dims).
Each core computes Q/K projections in transposed layout (head-dim on
partitions), V natural, attention without max subtraction (scores ~ N(0,1) in
fp32), the softmax denominator via a ones-column appended to V (free inside
the ctx matmul M=65), and a row-parallel partial of the output projection.
The host sums the 4 partials per batch and adds the output bias.

HW-measured matmul design rules (see microbenches):
  - back-to-back matmuls on the SAME PE row group cost ~449ns (LDWEIGHTS
    serializes); alternating row groups or full-K with bank alternation gets
    200-255ns (LDW hides behind the other stream).
  - accumulation chains into the same PSUM bank back-to-back cost ~365ns;
    interleaving two chains on different banks avoids the hazard.
So every matmul stream below interleaves two chains (alternating PSUM banks)
and, for K=64 score matmuls, alternates row groups via the hi (head) index.

The exp runs as [128, 1024] activations over 2-bank PSUM score tiles to
amortize the ~143ns per-instruction ACT overhead (ACT is ~128us total and
near-critical).
"""

import numpy as np

import concourse.tile as tile
from concourse import bacc, mybir
from concourse import bass_utils

EMB = 1024
S = 2048
B = 2
HPC = 4            # heads per core
DQ = HPC * 64      # 256 projection dims per core
NCORES = 8

F32 = mybir.dt.float32
F32R = mybir.dt.float32r
EXP = mybir.ActivationFunctionType.Exp

KT_E = EMB // 128  # 8 contraction tiles over EMB
NQC = S // 512     # 4 query chunks
NST = S // 128     # 16 sequence tiles

_NC = None
TRACE = False
LAST_RESULT = None


def _mha(ctx, tc, xqT, xkT, xvT, wqT, wkT, wvT, woT, bq, bk, bv, out, bench_iters=None):
    nc = tc.nc

    cstp = ctx.enter_context(tc.tile_pool(name="const", bufs=1))
    xpool = ctx.enter_context(tc.tile_pool(name="xin", bufs=16))
    epool = ctx.enter_context(tc.tile_pool(name="exp", bufs=6))
    bpool = ctx.enter_context(tc.tile_pool(name="bcsb", bufs=2))
    opool = ctx.enter_context(tc.tile_pool(name="osb", bufs=3))
    upool = ctx.enter_context(tc.tile_pool(name="unorm", bufs=8))
    # PSUM: spool 4x[128,512] (4 banks, shared by scores / projections /
    # out-proj / bcast) + ctx_ps 4x[65,512] (4 banks) = 8 banks exactly.
    spool = ctx.enter_context(tc.tile_pool(name="scps", bufs=4, space="PSUM"))
    ctx_ps = ctx.enter_context(tc.tile_pool(name="ctxps", bufs=4, space="PSUM"))
    mpool = spool

    # ---- persistent SBUF tensors ----
    ones_row = cstp.tile([1, 512], F32R)
    nc.vector.memset(ones_row[:].bitcast(F32), 1.0)
    sel64 = cstp.tile([65, 64], F32R)           # one-hot: row 64 -> all cols
    nc.vector.memset(sel64[:].bitcast(F32), 0.0)
    nc.vector.memset(sel64[64:65, :].bitcast(F32), 1.0)
    rden = cstp.tile([65, 512], F32R)           # row 64 = 1/denom, rows 0..63 zero
    nc.vector.memset(rden[0:64, :].bitcast(F32), 0.0)

    wq_sb = cstp.tile([128, KT_E * DQ], F32R)   # [128, 2048]: wq_sb[p, n*256+m] = WqT[n*128+p, m]
    wk_sb = cstp.tile([128, KT_E * DQ], F32R)
    wv_sb = cstp.tile([128, KT_E * DQ], F32R)
    for sb, src in ((wq_sb, wqT), (wk_sb, wkT), (wv_sb, wvT)):
        nc.sync.dma_start(
            sb[:].rearrange("p (n m) -> p n m", n=KT_E),
            src.rearrange("(n p) m -> p n m", p=128),
        )
    wo_sb = cstp.tile([128, 2 * EMB], F32R)     # wo_sb[p, n*1024+f] = WoT[n*128+p, f]
    nc.sync.dma_start(
        wo_sb[:].rearrange("p (n m) -> p n m", n=2),
        woT.rearrange("(n p) m -> p n m", p=128),
    )
    # per-partition bias columns: b?c[p, dq] = bias[dq*128+p]
    bqc = cstp.tile([128, 2], F32)
    bkc = cstp.tile([128, 2], F32)
    for sb, src in ((bqc, bq), (bkc, bk)):
        nc.sync.dma_start(sb[:], src.rearrange("o (d p) -> p (o d)", p=128))
    bv_sb = cstp.tile([1, DQ], F32R)
    nc.sync.dma_start(bv_sb[:], bv[:])

    # results of phase 1 kept resident
    kT_sb = cstp.tile([128, 2 * S], F32R)       # [dq-block 2][s 2048]
    qT_sb = cstp.tile([128, 2 * S], F32R)
    ctxT_sb = cstp.tile([128, 2 * S], F32R)
    v_sb = cstp.tile([128, NST * (HPC * 65)], F32R)  # per s-tile: 4 heads x (64 V + ones col)
    nc.vector.memset(
        v_sb[:].bitcast(F32).rearrange("p (t h m) -> p t h m", t=NST, h=HPC)[:, :, :, 64:65],
        1.0,
    )

    def body():
        _body(tc, nc, xqT, xkT, xvT, out, ones_row, sel64, rden, wq_sb, wk_sb,
              wv_sb, wo_sb, bqc, bkc, bv_sb, kT_sb, qT_sb, ctxT_sb, v_sb,
              xpool, epool, bpool, opool, upool, mpool, spool, ctx_ps)

    if bench_iters:
        hints = (
            mybir.EngineType.PE,
            mybir.EngineType.Activation,
            mybir.EngineType.DVE,
            mybir.EngineType.SP,
            mybir.EngineType.Pool,
        )
        with tc.For_i(0, bench_iters, 1, hint_engines=hints):
            body()
    else:
        body()


def _body(tc, nc, xqT, xkT, xvT, out, ones_row, sel64, rden, wq_sb, wk_sb,
          wv_sb, wo_sb, bqc, bkc, bv_sb, kT_sb, qT_sb, ctxT_sb, v_sb,
          xpool, epool, bpool, opool, upool, mpool, spool, ctx_ps):
    pending_norm = []

    def _norm_and_outproj(qc_done):
        _finish_chunk(nc, qc_done, pending_norm, rden, sel64, bpool,
                      opool, mpool, ctxT_sb, wo_sb, out)

    # ---- phase 1: per sequence chunk, produce Q^T, K^T and V ----
    for qc in range(NQC):
        for pj, (w_sb, xsrc, dst_sb, bcol) in enumerate((
            (wq_sb, xqT, qT_sb, bqc),
            (wk_sb, xkT, kT_sb, bkc),
        )):
            xs = []
            for kt in range(KT_E):
                t = xpool.tile([128, 512], F32R, tag="xchunk", name=f"x_{qc}_{pj}_{kt}")
                nc.sync.dma_start(t[:], xsrc[kt, qc])
                xs.append(t)
            # two dq chains interleaved -> PSUM banks alternate
            ps = [mpool.tile([128, 512], F32, tag="sc", name=f"p_{qc}_{pj}_{dq}")
                  for dq in range(2)]
            for kt in range(KT_E):
                for dq in range(2):
                    nc.tensor.matmul(
                        ps[dq][:], w_sb[:, kt * DQ + dq * 128: kt * DQ + dq * 128 + 128],
                        xs[kt][:], start=(kt == 0), stop=(kt == KT_E - 1),
                    )
            for dq in range(2):
                nc.vector.tensor_scalar_add(
                    dst_sb[:, dq * S + qc * 512: dq * S + qc * 512 + 512],
                    ps[dq][:], bcol[:, dq: dq + 1],
                )
        # V: natural layout, two chains at a time (padded [128,512] psum tiles
        # so the "mm" tag keeps a single slot size)
        xv = []
        for kt in range(KT_E):
            t = xpool.tile([128, 512], F32R, tag="xchunk", name=f"xv_{qc}_{kt}")
            nc.sync.dma_start(t[:], xvT[kt, qc])
            xv.append(t)
        for pair in range(2):
            vp = [mpool.tile([128, 512], F32, tag="sc", name=f"vps_{qc}_{pair}_{i}")
                  for i in range(2)]
            for kt in range(KT_E):
                for i in range(2):
                    sti = pair * 2 + i
                    nc.tensor.matmul(
                        vp[i][:, 0:DQ], xv[kt][:, sti * 128: sti * 128 + 128],
                        wv_sb[:, kt * DQ: kt * DQ + DQ],
                        start=(kt == 0), stop=False,
                    )
            for i in range(2):
                nc.tensor.matmul(
                    vp[i][:, 0:DQ], ones_row[0:1, 0:128], bv_sb[0:1, :],
                    start=False, stop=True,
                )
            for i in range(2):
                sti = pair * 2 + i
                st = qc * 4 + sti
                dst = v_sb[:, st * (HPC * 65): (st + 1) * (HPC * 65)]
                nc.vector.tensor_copy(
                    dst.rearrange("p (h m) -> p h m", h=HPC)[:, :, 0:64],
                    vp[i][:, 0:DQ].rearrange("p (h m) -> p h m", h=HPC),
                )

    # ---- phase 2: per query chunk: attention, then deferred norm/out-proj ----
    for qc in range(NQC):
        for hp in range(2):
            # four ctx accumulation chains per head-pair: [hi][half]; each
            # chain keeps a constant row-group position, consecutive matmuls
            # alternate both row groups and PSUM banks (measured ~125ns/mm).
            cps = [
                [ctx_ps.tile([65, 512], F32, tag="ctx", name=f"ctx_{qc}_{hp}_{hi}_{half}")
                 for half in range(2)]
                for hi in range(2)
            ]

            def ctx_mms(es, kt):
                for hi in range(2):
                    h = hp * 2 + hi
                    vcol = kt * (HPC * 65) + h * 65
                    for half, b in enumerate((0, 64)):
                        nc.tensor.matmul(
                            cps[hi][half][:], v_sb[b:b + 64, vcol: vcol + 65],
                            es[hi][b:b + 64, :],
                            start=(kt == 0), stop=(kt == NST - 1),
                        )

            prev = None
            for kt in range(NST):
                es = []
                for hi in range(2):
                    base = 64 * hi
                    blk = hp * S
                    sc = spool.tile([128, 512], F32, tag="sc", name=f"sc_{qc}_{hp}_{kt}_{hi}")
                    nc.tensor.matmul(
                        sc[:],
                        kT_sb[base:base + 64, blk + kt * 128: blk + kt * 128 + 128],
                        qT_sb[base:base + 64, blk + qc * 512: blk + qc * 512 + 512],
                        start=True, stop=True,
                    )
                    e = epool.tile([128, 512], F32R, tag="e", name=f"e_{qc}_{hp}_{kt}_{hi}")
                    nc.scalar.activation(e[:], sc[:], EXP, scale=0.125)
                    es.append(e)
                if prev is not None:
                    ctx_mms(*prev)
                prev = (es, kt)
            ctx_mms(*prev)
            for hi in range(2):
                # combine the two half-chains into SBUF; normalization +
                # out-projection are deferred one qc so they overlap the next
                # chunk's attention.
                tmpa = bpool.tile([65, 512], F32, tag="tmpa", name=f"tmpa_{qc}_{hp}_{hi}")
                nc.vector.tensor_copy(tmpa[:], cps[hi][0][:])
                u = upool.tile([65, 512], F32, tag="u", name=f"u_{qc}_{hp}_{hi}")
                nc.vector.tensor_add(u[:], cps[hi][1][:], tmpa[:])
                pending_norm.append((qc, hp, hi, u))

        if qc > 0:
            _norm_and_outproj(qc - 1)
    _norm_and_outproj(NQC - 1)


def _finish_chunk(nc, qc, pending_norm, rden, sel64, bpool, opool, mpool,
                  ctxT_sb, wo_sb, out):
    for pqc, hp, hi, u in [p for p in pending_norm if p[0] == qc]:
        # reciprocal of the raw denominator row, broadcast to 64 partitions
        # via one-hot matmul, then multiply (normalizes ctx into ctxT_sb)
        with nc.allow_low_precision(reason="f32r is 32-bit; rounding only"):
            nc.vector.reciprocal(rden[64:65, :], u[64:65, :])
        bps = mpool.tile([128, 512], F32, tag="sc", name=f"bc_{qc}_{hp}_{hi}")
        nc.tensor.matmul(bps[0:64, :], sel64[:], rden[:], start=True, stop=True)
        nc.vector.tensor_mul(
            ctxT_sb[64 * hi: 64 * hi + 64, hp * S + qc * 512: hp * S + qc * 512 + 512],
            u[0:64, :],
            bps[0:64, :],
        )
    # out projection for this chunk's 4 query tiles; fc chains interleaved
    for qt4 in range(4):
        qt = qc * 4 + qt4
        ps = [mpool.tile([128, 512], F32, tag="sc", name=f"ops_{qt}_{fc}")
              for fc in range(2)]
        for dq in range(2):
            for fc in range(2):
                nc.tensor.matmul(
                    ps[fc][:],
                    ctxT_sb[:, dq * S + qt * 128: dq * S + qt * 128 + 128],
                    wo_sb[:, dq * EMB + fc * 512: dq * EMB + fc * 512 + 512],
                    start=(dq == 0), stop=(dq == 1),
                )
        ot = opool.tile([128, EMB], F32, tag="o", name=f"ot_{qt}")
        for fc in range(2):
            nc.vector.tensor_copy(ot[:, fc * 512: fc * 512 + 512], ps[fc][:])
        nc.gpsimd.dma_start(out[qt * 128:(qt + 1) * 128, :], ot[:])


def _build_nc(bench_iters=None):
    from contextlib import ExitStack

    nc = bacc.Bacc("TRN2", target_bir_lowering=False, debug=False, num_devices=NCORES)
    xqT = nc.dram_tensor("xqT", [KT_E, NQC, 128, 512], F32R, kind="ExternalInput").ap()
    xkT = nc.dram_tensor("xkT", [KT_E, NQC, 128, 512], F32R, kind="ExternalInput").ap()
    xvT = nc.dram_tensor("xvT", [KT_E, NQC, 128, 512], F32R, kind="ExternalInput").ap()
    wqT = nc.dram_tensor("wqT", [EMB, DQ], F32R, kind="ExternalInput").ap()
    wkT = nc.dram_tensor("wkT", [EMB, DQ], F32R, kind="ExternalInput").ap()
    wvT = nc.dram_tensor("wvT", [EMB, DQ], F32R, kind="ExternalInput").ap()
    woT = nc.dram_tensor("woT", [DQ, EMB], F32R, kind="ExternalInput").ap()
    bq = nc.dram_tensor("bq", [1, DQ], F32, kind="ExternalInput").ap()
    bk = nc.dram_tensor("bk", [1, DQ], F32, kind="ExternalInput").ap()
    bv = nc.dram_tensor("bv", [1, DQ], F32R, kind="ExternalInput").ap()
    out = nc.dram_tensor("out", [S, EMB], F32, kind="ExternalOutput").ap()

    with ExitStack() as ctx:
        tc = ctx.enter_context(tile.TileContext(nc))
        _mha(ctx, tc, xqT, xkT, xvT, wqT, wkT, wvT, woT, bq, bk, bv, out,
             bench_iters=bench_iters)
    nc.compile()
    return nc


def _chunk_major(x):
    """[S, EMB] -> x.T chunked as [KT_E, NQC, 128, 512] (each chunk contiguous)."""
    xt = x.T  # [EMB, S]
    return np.ascontiguousarray(
        xt.reshape(KT_E, 128, NQC, 512).transpose(0, 2, 1, 3)
    )


def kernel(query, key, value, Wq, bq, Wk, bk, Wv, bv, Wo, bo):
    global _NC, LAST_RESULT
    query, key, value, Wq, bq, Wk, bk, Wv, bv, Wo, bo = (
        np.asarray(a, dtype=np.float32)
        for a in (query, key, value, Wq, bq, Wk, bk, Wv, bv, Wo, bo)
    )
    if _NC is None:
        _NC = _build_nc()

    in_maps = []
    for c in range(NCORES):
        b, g = divmod(c, 4)
        rows = slice(g * DQ, (g + 1) * DQ)
        in_maps.append({
            "xqT": _chunk_major(query[b]),
            "xkT": _chunk_major(key[b]),
            "xvT": _chunk_major(value[b]),
            "wqT": np.ascontiguousarray(Wq[rows].T),
            "wkT": np.ascontiguousarray(Wk[rows].T),
            "wvT": np.ascontiguousarray(Wv[rows].T),
            "woT": np.ascontiguousarray(Wo[:, rows].T),
            "bq": np.ascontiguousarray(bq[rows][None, :]),
            "bk": np.ascontiguousarray(bk[rows][None, :]),
            "bv": np.ascontiguousarray(bv[rows][None, :]),
        })

    res = bass_utils.run_bass_kernel_spmd(
        _NC, in_maps, core_ids=list(range(NCORES)), trace=TRACE
    )
    LAST_RESULT = res

    out = np.zeros((B, S, EMB), np.float32)
    for c in range(NCORES):
        out[c // 4] += res.results[c]["out"]
    out += bo[None, None, :]
    return out


# revision 13
# speedup vs baseline: 1.1919x; 1.0311x over previous
"""Multi-head attention (B=2, S=2048, EMB=1024, 16 heads) on 8 Trainium2 cores.

Sharding: core c -> batch c//4, head-group c%4 (4 heads = 256 projection dims).
Each core computes Q/K projections in transposed layout (head-dim on
partitions), V natural, attention without max subtraction (scores ~ N(0,1) in
fp32), the softmax denominator via a ones-column appended to V (free inside
the ctx matmul M=65), and a row-parallel partial of the output projection.
The host sums the 4 partials per batch and adds the output bias.

HW-measured matmul design rules (see microbenches):
  - back-to-back matmuls on the SAME PE row group cost ~449ns (LDWEIGHTS
    serializes); alternating row groups or full-K with bank alternation gets
    200-255ns (LDW hides behind the other stream).
  - accumulation chains into the same PSUM bank back-to-back cost ~365ns;
    interleaving two chains on different banks avoids the hazard.
So every matmul stream below interleaves two chains (alternating PSUM banks)
and, for K=64 score matmuls, alternates row groups via the hi (head) index.

The exp runs as [128, 1024] activations over 2-bank PSUM score tiles to
amortize the ~143ns per-instruction ACT overhead (ACT is ~128us total and
near-critical).
"""

import numpy as np

import concourse.tile as tile
from concourse import bacc, mybir
from concourse import bass_utils

EMB = 1024
S = 2048
B = 2
HPC = 4            # heads per core
DQ = HPC * 64      # 256 projection dims per core
NCORES = 8

F32 = mybir.dt.float32
F32R = mybir.dt.float32r
EXP = mybir.ActivationFunctionType.Exp

KT_E = EMB // 128  # 8 contraction tiles over EMB
NQC = S // 512     # 4 query chunks
NST = S // 128     # 16 sequence tiles

_NC = None
TRACE = False
LAST_RESULT = None
STAGE = "full"     # "proj" | "sc" | "ctx" | "full" (cumulative)


def _mha(ctx, tc, xqT, xkT, xvT, wqT, wkT, wvT, woT, bq, bk, bv, out, bench_iters=None):
    nc = tc.nc

    cstp = ctx.enter_context(tc.tile_pool(name="const", bufs=1))
    xpool = ctx.enter_context(tc.tile_pool(name="xin", bufs=16))
    epool = ctx.enter_context(tc.tile_pool(name="exp", bufs=6))
    bpool = ctx.enter_context(tc.tile_pool(name="bcsb", bufs=2))
    opool = ctx.enter_context(tc.tile_pool(name="osb", bufs=3))
    upool = ctx.enter_context(tc.tile_pool(name="unorm", bufs=8))
    # PSUM: spool 2x[128,512] (scores; 2 banks, ACT-paced double buffer)
    # + ctx_ps 4x[65,512] (4 banks) + mpool 2x[128,512] (projections /
    # out-proj / bcast; 2 banks) = 8 banks exactly.
    spool = ctx.enter_context(tc.tile_pool(name="scps", bufs=2, space="PSUM"))
    ctx_ps = ctx.enter_context(tc.tile_pool(name="ctxps", bufs=4, space="PSUM"))
    mpool = ctx.enter_context(tc.tile_pool(name="mmps", bufs=2, space="PSUM"))

    # ---- persistent SBUF tensors ----
    ones_row = cstp.tile([1, 512], F32R)
    nc.vector.memset(ones_row[:].bitcast(F32), 1.0)
    sel64 = cstp.tile([65, 64], F32R)           # one-hot: row 64 -> all cols
    nc.vector.memset(sel64[:].bitcast(F32), 0.0)
    nc.vector.memset(sel64[64:65, :].bitcast(F32), 1.0)
    rden = cstp.tile([65, 512], F32R)           # row 64 = 1/denom, rows 0..63 zero
    nc.vector.memset(rden[0:64, :].bitcast(F32), 0.0)

    wq_sb = cstp.tile([128, KT_E * DQ], F32R)   # [128, 2048]: wq_sb[p, n*256+m] = WqT[n*128+p, m]
    wk_sb = cstp.tile([128, KT_E * DQ], F32R)
    wv_sb = cstp.tile([128, KT_E * DQ], F32R)
    for sb, src in ((wq_sb, wqT), (wk_sb, wkT), (wv_sb, wvT)):
        nc.sync.dma_start(
            sb[:].rearrange("p (n m) -> p n m", n=KT_E),
            src.rearrange("(n p) m -> p n m", p=128),
        )
    wo_sb = cstp.tile([128, 2 * EMB], F32R)     # wo_sb[p, n*1024+f] = WoT[n*128+p, f]
    nc.sync.dma_start(
        wo_sb[:].rearrange("p (n m) -> p n m", n=2),
        woT.rearrange("(n p) m -> p n m", p=128),
    )
    # per-partition bias columns: b?c[p, dq] = bias[dq*128+p]
    bqc = cstp.tile([128, 2], F32)
    bkc = cstp.tile([128, 2], F32)
    for sb, src in ((bqc, bq), (bkc, bk)):
        nc.sync.dma_start(sb[:], src.rearrange("o (d p) -> p (o d)", p=128))
    bv_sb = cstp.tile([1, DQ], F32R)
    nc.sync.dma_start(bv_sb[:], bv[:])

    # results of phase 1 kept resident
    kT_sb = cstp.tile([128, 2 * S], F32R)       # [dq-block 2][s 2048]
    qT_sb = cstp.tile([128, 2 * S], F32R)
    ctxT_sb = cstp.tile([128, 2 * S], F32R)
    v_sb = cstp.tile([128, NST * (HPC * 65)], F32R)  # per s-tile: 4 heads x (64 V + ones col)
    nc.vector.memset(
        v_sb[:].bitcast(F32).rearrange("p (t h m) -> p t h m", t=NST, h=HPC)[:, :, :, 64:65],
        1.0,
    )

    def body():
        _body(tc, nc, xqT, xkT, xvT, out, ones_row, sel64, rden, wq_sb, wk_sb,
              wv_sb, wo_sb, bqc, bkc, bv_sb, kT_sb, qT_sb, ctxT_sb, v_sb,
              xpool, epool, bpool, opool, upool, mpool, spool, ctx_ps)

    if bench_iters:
        hints = (
            mybir.EngineType.PE,
            mybir.EngineType.Activation,
            mybir.EngineType.DVE,
            mybir.EngineType.SP,
            mybir.EngineType.Pool,
        )
        with tc.For_i(0, bench_iters, 1, hint_engines=hints):
            body()
    else:
        body()


def _body(tc, nc, xqT, xkT, xvT, out, ones_row, sel64, rden, wq_sb, wk_sb,
          wv_sb, wo_sb, bqc, bkc, bv_sb, kT_sb, qT_sb, ctxT_sb, v_sb,
          xpool, epool, bpool, opool, upool, mpool, spool, ctx_ps):
    pending_norm = []
    fin_slices = []

    def _queue_finish(qc_done):
        fin_slices.extend(_finish_slices(nc, qc_done, pending_norm, rden, sel64,
                                         opool, mpool, ctxT_sb, wo_sb, out))

    def _pop_finish():
        if fin_slices:
            fin_slices.pop(0)()

    # ---- phase 1: per sequence chunk, produce Q^T, K^T and V ----
    for qc in range(NQC):
        for pj, (w_sb, xsrc, dst_sb, bcol) in enumerate((
            (wq_sb, xqT, qT_sb, bqc),
            (wk_sb, xkT, kT_sb, bkc),
        )):
            xs = []
            for kt in range(KT_E):
                t = xpool.tile([128, 512], F32R, tag="xchunk", name=f"x_{qc}_{pj}_{kt}")
                nc.sync.dma_start(t[:], xsrc[kt, qc])
                xs.append(t)
            # two dq chains interleaved -> PSUM banks alternate
            ps = [mpool.tile([128, 512], F32, tag="mm", name=f"p_{qc}_{pj}_{dq}")
                  for dq in range(2)]
            for kt in range(KT_E):
                for dq in range(2):
                    nc.tensor.matmul(
                        ps[dq][:], w_sb[:, kt * DQ + dq * 128: kt * DQ + dq * 128 + 128],
                        xs[kt][:], start=(kt == 0), stop=(kt == KT_E - 1),
                    )
            for dq in range(2):
                nc.vector.tensor_scalar_add(
                    dst_sb[:, dq * S + qc * 512: dq * S + qc * 512 + 512],
                    ps[dq][:], bcol[:, dq: dq + 1],
                )
        # V: natural layout, two chains at a time (padded [128,512] psum tiles
        # so the "mm" tag keeps a single slot size)
        xv = []
        for kt in range(KT_E):
            t = xpool.tile([128, 512], F32R, tag="xchunk", name=f"xv_{qc}_{kt}")
            nc.sync.dma_start(t[:], xvT[kt, qc])
            xv.append(t)
        for pair in range(2):
            vp = [mpool.tile([128, 512], F32, tag="mm", name=f"vps_{qc}_{pair}_{i}")
                  for i in range(2)]
            for kt in range(KT_E):
                for i in range(2):
                    sti = pair * 2 + i
                    nc.tensor.matmul(
                        vp[i][:, 0:DQ], xv[kt][:, sti * 128: sti * 128 + 128],
                        wv_sb[:, kt * DQ: kt * DQ + DQ],
                        start=(kt == 0), stop=False,
                    )
            for i in range(2):
                nc.tensor.matmul(
                    vp[i][:, 0:DQ], ones_row[0:1, 0:128], bv_sb[0:1, :],
                    start=False, stop=True,
                )
            for i in range(2):
                sti = pair * 2 + i
                st = qc * 4 + sti
                dst = v_sb[:, st * (HPC * 65): (st + 1) * (HPC * 65)]
                nc.vector.tensor_copy(
                    dst.rearrange("p (h m) -> p h m", h=HPC)[:, :, 0:64],
                    vp[i][:, 0:DQ].rearrange("p (h m) -> p h m", h=HPC),
                )

    # ---- phase 2: per query chunk: attention, then deferred norm/out-proj ----
    if STAGE == "proj":
        return
    for qc in range(NQC):
        for hp in range(2):
            # four ctx accumulation chains per head-pair: [hi][half]; each
            # chain keeps a constant row-group position, consecutive matmuls
            # alternate both row groups and PSUM banks (measured ~125ns/mm).
            cps = [
                [ctx_ps.tile([65, 512], F32, tag="ctx", name=f"ctx_{qc}_{hp}_{hi}_{half}")
                 for half in range(2)]
                for hi in range(2)
            ]

            def ctx_mms(es, kt):
                for hi in range(2):
                    h = hp * 2 + hi
                    vcol = kt * (HPC * 65) + h * 65
                    for half, b in enumerate((0, 64)):
                        nc.tensor.matmul(
                            cps[hi][half][:], v_sb[b:b + 64, vcol: vcol + 65],
                            es[hi][b:b + 64, :],
                            start=(kt == 0), stop=(kt == NST - 1),
                        )

            prev = None
            for kt in range(NST):
                es = []
                for hi in range(2):
                    base = 64 * hi
                    blk = hp * S
                    sc = spool.tile([128, 512], F32, tag="sc", name=f"sc_{qc}_{hp}_{kt}_{hi}")
                    nc.tensor.matmul(
                        sc[:],
                        kT_sb[base:base + 64, blk + kt * 128: blk + kt * 128 + 128],
                        qT_sb[base:base + 64, blk + qc * 512: blk + qc * 512 + 512],
                        start=True, stop=True,
                    )
                    e = epool.tile([128, 512], F32R, tag="e", name=f"e_{qc}_{hp}_{kt}_{hi}")
                    nc.scalar.activation(e[:], sc[:], EXP, scale=0.125)
                    es.append(e)
                if prev is not None and STAGE != "sc":
                    ctx_mms(*prev)
                prev = (es, kt)
                if kt % 4 == 2:
                    # interleave one slice of the previous chunk's deferred
                    # normalization / out-projection into the ACT-bound loop
                    _pop_finish()
            if STAGE != "sc":
                ctx_mms(*prev)
            for hi in range(2 if STAGE not in ("sc",) else 0):
                # combine the two half-chains into SBUF; normalization +
                # out-projection are deferred one qc so they overlap the next
                # chunk's attention.
                tmpa = bpool.tile([65, 512], F32, tag="tmpa", name=f"tmpa_{qc}_{hp}_{hi}")
                nc.vector.tensor_copy(tmpa[:], cps[hi][0][:])
                u = upool.tile([65, 512], F32, tag="u", name=f"u_{qc}_{hp}_{hi}")
                nc.vector.tensor_add(u[:], cps[hi][1][:], tmpa[:])
                pending_norm.append((qc, hp, hi, u))

        if STAGE == "full":
            _queue_finish(qc)
    while fin_slices:
        _pop_finish()


def _finish_slices(nc, qc, pending_norm, rden, sel64, opool, mpool,
                   ctxT_sb, wo_sb, out):
    """Return the deferred norm + out-projection work for chunk qc as a list
    of closures, so the caller can interleave them into the next chunk's
    ACT-bound attention loop (PE/DVE fill the slack there)."""
    slices = []

    def norm_slice(hp, hi, u):
        def go():
            # reciprocal of the raw denominator row, broadcast to 64
            # partitions via one-hot matmul, then multiply into ctxT_sb
            with nc.allow_low_precision(reason="f32r is 32-bit; rounding only"):
                nc.vector.reciprocal(rden[64:65, :], u[64:65, :])
            bps = mpool.tile([128, 512], F32, tag="mm", name=f"bc_{qc}_{hp}_{hi}")
            nc.tensor.matmul(bps[0:64, :], sel64[:], rden[:], start=True, stop=True)
            nc.vector.tensor_mul(
                ctxT_sb[64 * hi: 64 * hi + 64, hp * S + qc * 512: hp * S + qc * 512 + 512],
                u[0:64, :],
                bps[0:64, :],
            )
        return go

    def out_slice(qt):
        def go():
            ps = [mpool.tile([128, 512], F32, tag="mm", name=f"ops_{qt}_{fc}")
                  for fc in range(2)]
            for dq in range(2):
                for fc in range(2):
                    nc.tensor.matmul(
                        ps[fc][:],
                        ctxT_sb[:, dq * S + qt * 128: dq * S + qt * 128 + 128],
                        wo_sb[:, dq * EMB + fc * 512: dq * EMB + fc * 512 + 512],
                        start=(dq == 0), stop=(dq == 1),
                    )
            ot = opool.tile([128, EMB], F32, tag="o", name=f"ot_{qt}")
            for fc in range(2):
                nc.vector.tensor_copy(ot[:, fc * 512: fc * 512 + 512], ps[fc][:])
            nc.gpsimd.dma_start(out[qt * 128:(qt + 1) * 128, :], ot[:])
        return go

    for pqc, hp, hi, u in [p for p in pending_norm if p[0] == qc]:
        slices.append(norm_slice(hp, hi, u))
    for qt4 in range(4):
        slices.append(out_slice(qc * 4 + qt4))
    return slices


def _build_nc(bench_iters=None):
    from contextlib import ExitStack

    nc = bacc.Bacc("TRN2", target_bir_lowering=False, debug=False, num_devices=NCORES)
    xqT = nc.dram_tensor("xqT", [KT_E, NQC, 128, 512], F32R, kind="ExternalInput").ap()
    xkT = nc.dram_tensor("xkT", [KT_E, NQC, 128, 512], F32R, kind="ExternalInput").ap()
    xvT = nc.dram_tensor("xvT", [KT_E, NQC, 128, 512], F32R, kind="ExternalInput").ap()
    wqT = nc.dram_tensor("wqT", [EMB, DQ], F32R, kind="ExternalInput").ap()
    wkT = nc.dram_tensor("wkT", [EMB, DQ], F32R, kind="ExternalInput").ap()
    wvT = nc.dram_tensor("wvT", [EMB, DQ], F32R, kind="ExternalInput").ap()
    woT = nc.dram_tensor("woT", [DQ, EMB], F32R, kind="ExternalInput").ap()
    bq = nc.dram_tensor("bq", [1, DQ], F32, kind="ExternalInput").ap()
    bk = nc.dram_tensor("bk", [1, DQ], F32, kind="ExternalInput").ap()
    bv = nc.dram_tensor("bv", [1, DQ], F32R, kind="ExternalInput").ap()
    out = nc.dram_tensor("out", [S, EMB], F32, kind="ExternalOutput").ap()

    with ExitStack() as ctx:
        tc = ctx.enter_context(tile.TileContext(nc))
        _mha(ctx, tc, xqT, xkT, xvT, wqT, wkT, wvT, woT, bq, bk, bv, out,
             bench_iters=bench_iters)
    nc.compile()
    return nc


def _chunk_major(x):
    """[S, EMB] -> x.T chunked as [KT_E, NQC, 128, 512] (each chunk contiguous)."""
    xt = x.T  # [EMB, S]
    return np.ascontiguousarray(
        xt.reshape(KT_E, 128, NQC, 512).transpose(0, 2, 1, 3)
    )


def kernel(query, key, value, Wq, bq, Wk, bk, Wv, bv, Wo, bo):
    global _NC, LAST_RESULT
    query, key, value, Wq, bq, Wk, bk, Wv, bv, Wo, bo = (
        np.asarray(a, dtype=np.float32)
        for a in (query, key, value, Wq, bq, Wk, bk, Wv, bv, Wo, bo)
    )
    if _NC is None:
        _NC = _build_nc()

    in_maps = []
    for c in range(NCORES):
        b, g = divmod(c, 4)
        rows = slice(g * DQ, (g + 1) * DQ)
        in_maps.append({
            "xqT": _chunk_major(query[b]),
            "xkT": _chunk_major(key[b]),
            "xvT": _chunk_major(value[b]),
            "wqT": np.ascontiguousarray(Wq[rows].T),
            "wkT": np.ascontiguousarray(Wk[rows].T),
            "wvT": np.ascontiguousarray(Wv[rows].T),
            "woT": np.ascontiguousarray(Wo[:, rows].T),
            "bq": np.ascontiguousarray(bq[rows][None, :]),
            "bk": np.ascontiguousarray(bk[rows][None, :]),
            "bv": np.ascontiguousarray(bv[rows][None, :]),
        })

    res = bass_utils.run_bass_kernel_spmd(
        _NC, in_maps, core_ids=list(range(NCORES)), trace=TRACE
    )
    LAST_RESULT = res

    out = np.zeros((B, S, EMB), np.float32)
    for c in range(NCORES):
        out[c // 4] += res.results[c]["out"]
    out += bo[None, None, :]
    return out
